# revision 1
# baseline (speedup 1.0000x reference)
"""Bass/Trainium2 kernel for nn_CapLayer (dynamic-routing capsule layer).

Key algebraic identity (holds for ANY x, W — verified against the reference):
the routing logits b start at zero; softmax over the out-caps axis of an
o-constant tensor is uniform (1/NUM_OUT); with uniform c the squashed v is
identical for every out-cap o, which makes delta_b = <pred, v> o-constant as
well, so b stays o-constant through every routing iteration and the softmax
stays uniform forever.  Hence:

    sbar[b, :] = (1/64) * sum_n pred[b, n, :]
               = (1/64) * sum_{s,i} (sum_p u[b,s,p,i]) * W[s,:,i]
    v[b, o, :] = sbar[b,:] * (|sbar| / (1 + |sbar|^2))     for all o.

So the kernel is: a full reduction of x over the per-group spatial axis
(memory bound — must read all of x exactly once), a tiny matmul with a
rearranged W, a squash, and a broadcast store.  Data-parallel over batch
across 8 cores.

On-chip dataflow per core (8 batches):
  - DMA d loads channel-block set J_d for ALL 8 batches (so downstream group
    results complete per-channel-block and overlap later DMAs).
  - DVE: segmented reduce over the 8 spatial repeats: [128c, 256] -> [128c, 32i]
  - PE (A2): lhsT=t[128c,32i], rhs=selector[128c,4g] -> psum[32i, b, 4s]
    (usum lands already transposed, i on partitions)
  - PE (B): per group s: lhsT=u2[32i, 8b], rhs=WT[32i, 64o] accumulating
    into psum sbar64[8b, 64o]; interleaved with later A2s.
  - squash epilogue on [8,64], broadcast store over the out-caps axis.
"""

import json

import numpy as np

import concourse.bass as bass
import concourse.tile as tile
from concourse import mybir
from concourse.bass_utils import run_bass_kernel_spmd

N_CORES = 8
BS = 64
BPC = BS // N_CORES  # 8 batches per core
NCH = 1024           # num_shared * in_dim channels
HW = 256             # 16*16 spatial
NS = 32              # num shared groups
IN_DIM = 32
OUT_DIM = 64
NUM_OUT = 64
F32 = mybir.dt.float32

N_DMA = 8            # x-shard loads per core (channel-block granularity)

# stash of the last run's BassKernelResults for test harnesses
LAST_RESULTS = None
_NC_CACHE = None


def _split_multi_waits(bir: bytes) -> bytes:
    """The walrus build in this toolchain only accepts a single sync-wait
    command per instruction; Tile freely attaches several (most notably the
    kernel-tail drain, which waits on every outstanding semaphore).  Rewrite
    the BIR so any instruction with N>1 waits is preceded by N-1 single-wait
    NoOps on the same engine — semantically identical (the engine stalls at
    the nops), and acceptable to this codegen."""
    j = json.loads(bir)
    ctr = [0]

    def fix_block(b):
        new = []
        for inst in b.get("instructions", []):
            si = inst.get("sync_info")
            if si:
                waits = si.get("on_wait") or []
                if len(waits) > 1:
                    for w in waits[:-1]:
                        ctr[0] += 1
                        new.append({
                            "debug": inst.get("debug", 0),
                            "engine": inst["engine"],
                            "ins": [],
                            "name": f"W-{ctr[0]}",
                            "opcode": "NoOp",
                            "outs": [],
                            "sync_info": {"on_update": [], "on_wait": [w]},
                        })
                    si["on_wait"] = [waits[-1]]
            new.append(inst)
        b["instructions"] = new
        for sb in b.get("blocks", []):
            fix_block(sb)

    for f in j.get("functions", []):
        for b in f.get("blocks", []):
            fix_block(b)
    return json.dumps(j).encode()


def _build(n_dma: int = N_DMA, probe: str = ""):
    assert 8 % (n_dma // 1) == 0 or n_dma in (1, 2, 4, 8)
    jblocks_per_dma = 8 // n_dma  # channel blocks (of 128) per DMA
    lvl = {"dma": 1, "reduce": 2, "a2": 3, "b": 4}.get(probe, 5)

    nc = bass.Bass()
    x = nc.dram_tensor("x", [BPC, NCH, HW], F32, kind="ExternalInput")
    wt = nc.dram_tensor("wt", [IN_DIM, NS, OUT_DIM], F32, kind="ExternalInput")
    # the out-caps axis of v is mathematically degenerate (identical for all
    # o) — the device emits only the unique [b, d] rows; the host unshard
    # step broadcasts to the full [b, o, d] shape.
    out = nc.dram_tensor("out", [BPC, OUT_DIM], F32, kind="ExternalOutput")

    with tile.TileContext(nc) as tc:
        with (
            tc.tile_pool(name="consts", bufs=1) as consts,
            tc.tile_pool(name="xp", bufs=17) as xp,
            tc.tile_pool(name="tp", bufs=8) as tp,
            tc.tile_pool(name="ep", bufs=1) as ep,
            tc.tile_pool(name="pp", bufs=1, space="PSUM") as pp,
        ):
            # constants: rearranged weights WT[i, s, o] = W[s, o, i], and the
            # group-selector matrix sel[c, g] = (c // 32 == g).  Matmul
            # operands are all produced by DVE so PE instructions carry a
            # single cross-engine wait.
            wt_stage = consts.tile([IN_DIM, NS, OUT_DIM], F32)
            nc.gpsimd.dma_start(out=wt_stage, in_=wt[:])
            wt_sb = consts.tile([IN_DIM, NS, OUT_DIM], F32)
            nc.vector.tensor_copy(out=wt_sb, in_=wt_stage)
            sel_sb = consts.tile([128, 4], F32)
            nc.vector.memset(sel_sb, 0.0)
            for g in range(4):
                nc.vector.memset(sel_sb[32 * g:32 * (g + 1), g:g + 1], 1.0)

            # u2[i, b, s] = usum[b, s, i]: accumulated straight out of PE
            u2 = pp.tile([IN_DIM, BPC, NS], F32)
            u2_sb = ep.tile([IN_DIM, BPC, NS], F32)
            sbar_ps = pp.tile([BPC, OUT_DIM], F32)

            # xv[p, j, b, m] = x[b, j*128 + p, m]
            xv = x.rearrange("b (j p) m -> p j b m", p=128)

            # Chunks = (channel block j, batch range [b0, b1)).  DVE reduces
            # run at ~2.2µs/MB vs DMA arrivals at ~2.9µs/MB, but a reduce can
            # only start at its chunk's completion semaphore — large chunks
            # make the reduce pipeline lag arrivals by a full chunk.  Shrink
            # chunks toward the end (halves -> quarters -> eighths) so the
            # DVE tracks the stream and the post-stream tail is minimal.
            chunks = []
            for j in range(6):
                chunks += [(j, 0, 4), (j, 4, 8)]
            chunks += [(6, 0, 2), (6, 2, 4), (6, 4, 6), (6, 6, 8)]
            chunks += [(7, 0, 2), (7, 2, 4), (7, 4, 6), (7, 6, 7), (7, 7, 8)]
            tks = {}
            for (j, b0, b1) in chunks:
                nb = b1 - b0
                xt = xp.tile([128, nb, HW], F32, tag="xt", name=f"xt_{j}_{b0}")
                nc.sync.dma_start(out=xt, in_=xv[:, j, b0:b1, :])
                if lvl < 2:
                    continue
                if j not in tks:
                    tks[j] = tp.tile([128, BPC, IN_DIM], F32, tag="tk",
                                     name=f"tk_{j}")
                tk = tks[j]
                # spatial m = k*32 + i ; reduce over the 8 k-repeats for all
                # batches of this chunk in one DVE op
                nc.vector.reduce_sum(
                    out=tk[:, b0:b1, :],
                    in_=xt.rearrange("p b (k i) -> p b i k", i=IN_DIM),
                    axis=mybir.AxisListType.X,
                )
                if lvl < 3:
                    continue
                for b in range(b0, b1):
                    # out[i, g] = sum_p tk[p, i] * sel[p, g];  s = 4j + g
                    nc.tensor.matmul(
                        out=u2[:, b, 4 * j:4 * j + 4],
                        lhsT=tk[:, b, :],
                        rhs=sel_sb[:],
                        start=True,
                        stop=True,
                        skip_group_check=True,
                    )
                if lvl < 4 or b1 != BPC:
                    continue
                # all 8 batches of groups 4j..4j+4 are now in PSUM: stage to
                # SBUF and run their B-matmuls immediately so they overlap
                # the remaining DMAs.
                nc.vector.tensor_copy(
                    out=u2_sb[:, :, 4 * j:4 * j + 4],
                    in_=u2[:, :, 4 * j:4 * j + 4],
                )
                for g in range(4):
                    s = 4 * j + g
                    # sbar[b, o] += sum_i usum[b,s,i] * W[s,o,i] / 64
                    nc.tensor.matmul(
                        out=sbar_ps,
                        lhsT=u2_sb[:, :, s],
                        rhs=wt_sb[:, s, :],
                        start=(s == 0),
                        stop=(s == NS - 1),
                        skip_group_check=True,
                    )

            if lvl < 5:
                dump = ep.tile([BPC, OUT_DIM], F32)
                nc.vector.memset(dump, 0.0)
                nc.sync.dma_start(out=out[:], in_=dump)
                orig_to_json_p = nc.to_json_bytes
                nc.to_json_bytes = lambda: _split_multi_waits(orig_to_json_p())
                return nc

            # squash on [8, 64]: v = sbar * coeff, coeff = sqrt(n2)/(1+n2),
            # n2 = |sbar|^2.  (wt is pre-scaled by 1/64 on the host, so
            # sbar_ps IS sbar.)  ACT fuses square+row-sum in one op reading
            # PSUM, then sqrt on the same engine; DVE does the reciprocal
            # chain and the final scale (also straight from PSUM).
            sq = ep.tile([BPC, OUT_DIM], F32)
            n2 = ep.tile([BPC, 1], F32)
            nc.scalar.activation(
                out=sq, in_=sbar_ps,
                func=mybir.ActivationFunctionType.Square,
                accum_out=n2,
            )
            r = ep.tile([BPC, 1], F32)
            nc.scalar.sqrt(out=r, in_=n2)
            d = ep.tile([BPC, 1], F32)
            nc.vector.tensor_scalar_add(out=d, in0=n2, scalar1=1.0)
            rd = ep.tile([BPC, 1], F32)
            nc.vector.reciprocal(out=rd, in_=d)
            coeff = ep.tile([BPC, 1], F32)
            nc.vector.tensor_mul(out=coeff, in0=r, in1=rd)
            vrow = ep.tile([BPC, OUT_DIM], F32)
            nc.vector.tensor_scalar_mul(out=vrow, in0=sbar_ps, scalar1=coeff)
            nc.sync.dma_start(out=out[:], in_=vrow)

    # every compile path (native walrus + bass2jax/axon) serializes via
    # to_json_bytes — splice the single-wait rewrite in there
    orig_to_json = nc.to_json_bytes
    nc.to_json_bytes = lambda: _split_multi_waits(orig_to_json())
    return nc


def kernel(x: np.ndarray, W: np.ndarray, trace: bool = False) -> np.ndarray:
    global LAST_RESULTS, _NC_CACHE
    x = np.ascontiguousarray(np.asarray(x, dtype=np.float32)).reshape(BS, NCH, HW)
    W = np.asarray(W, dtype=np.float32)

    # [i, s, o], pre-scaled so the PE B-stage directly produces sbar
    wt = np.ascontiguousarray(W.transpose(2, 0, 1)) * np.float32(1.0 / 64.0)

    if _NC_CACHE is None:
        _NC_CACHE = _build()
    nc = _NC_CACHE
    in_maps = [
        {"x": np.ascontiguousarray(x[c * BPC:(c + 1) * BPC]), "wt": wt}
        for c in range(N_CORES)
    ]
    res = run_bass_kernel_spmd(nc, in_maps, core_ids=list(range(N_CORES)), trace=trace)
    LAST_RESULTS = res
    rows = np.concatenate([r["out"] for r in res.results], axis=0)  # [64, 64]
    # unshard: materialize the degenerate out-caps axis (v is identical for
    # every o — see the module docstring)
    return np.ascontiguousarray(
        np.broadcast_to(rows[:, None, :], (BS, NUM_OUT, OUT_DIM))
    )



# revision 39
# speedup vs baseline: 1.0215x; 1.0215x over previous
"""Bass/Trainium2 kernel for nn_CapLayer (dynamic-routing capsule layer).

Key algebraic identity (holds for ANY x, W — verified against the reference):
the routing logits b start at zero; softmax over the out-caps axis of an
o-constant tensor is uniform (1/NUM_OUT); with uniform c the squashed v is
identical for every out-cap o, which makes delta_b = <pred, v> o-constant as
well, so b stays o-constant through every routing iteration and the softmax
stays uniform forever.  Hence:

    sbar[b, :] = (1/64) * sum_n pred[b, n, :]
               = (1/64) * sum_{s,i} (sum_p u[b,s,p,i]) * W[s,:,i]
    v[b, o, :] = sbar[b,:] * (|sbar| / (1 + |sbar|^2))     for all o.

So the kernel is: a full reduction of x over the per-group spatial axis
(memory bound — must read all of x exactly once), a tiny matmul with a
rearranged W, a squash, and a broadcast store.  Data-parallel over batch
across 8 cores.

On-chip dataflow per core (8 batches), channel-block (j) major streaming so
the single batched epilogue leaves almost nothing after the last DMA byte:

  - DMA chunks shrink toward the stream tail (halves -> quarters -> singles
    -> spatial halves) so the DVE reduce pipeline drains with the arrivals;
    the final chunk is half a block (64KB, ~194ns reduce).
  - DVE: segmented reduce over the 8 spatial repeats:
    [128c, nb, 256] -> [128c, nb, 32i]
  - PE (A2): per (block j, batch b): lhsT=tk[128c,32i], rhs=sel[128c,4g]
    -> psum u2[32i, b, 4s] (usum lands already transposed, i on partitions)
  - DVE: per block j: copy psum u2 slice -> SBUF as f16
  - PE (B): per group s: lhsT=u2_sb[32i, 8b] (f16), rhs=WT[32i, 64o] (f16)
    accumulating into psum sbar[8b, 64o]; f16 runs PE 4x faster than f32
    and W/usum quantization adds only ~1e-3 relative error.
  - squash epilogue on [8, 64]: Pool fuses square+row-sum (idle engine, no
    accumulator-read penalty), ACT does sqrt, DVE the reciprocal chain and
    the final two-scalar scale; one [8,64] store at the end.

The weights travel as f16 through the SWDGE (Pool) queue: half the DMA
bytes, desc-gen off the 8 HWDGE ring lanes, and no head-of-stream delay.
"""

import json

import numpy as np

import concourse.bass as bass
import concourse.tile as tile
from concourse import mybir
from concourse.alu_op_type import AluOpType
from concourse.bass_utils import run_bass_kernel_spmd

N_CORES = 8
BS = 64
BPC = BS // N_CORES  # 8 batches per core
NCH = 1024           # num_shared * in_dim channels
HW = 256             # 16*16 spatial
NS = 32              # num shared groups
IN_DIM = 32
OUT_DIM = 64
NUM_OUT = 64
F32 = mybir.dt.float32
F16 = mybir.dt.float16

# stash of the last run's BassKernelResults for test harnesses
LAST_RESULTS = None
_NC_CACHE = None


def _split_multi_waits(bir: bytes) -> bytes:
    """The walrus build in this toolchain only accepts a single sync-wait
    command per instruction; Tile freely attaches several (most notably the
    kernel-tail drain, which waits on every outstanding semaphore).  Rewrite
    the BIR so any instruction with N>1 waits is preceded by N-1 single-wait
    NoOps on the same engine — semantically identical (the engine stalls at
    the nops), and acceptable to this codegen."""
    j = json.loads(bir)
    ctr = [0]

    def fix_block(b):
        new = []
        for inst in b.get("instructions", []):
            si = inst.get("sync_info")
            if si:
                waits = si.get("on_wait") or []
                if len(waits) > 1:
                    for w in waits[:-1]:
                        ctr[0] += 1
                        new.append({
                            "debug": inst.get("debug", 0),
                            "engine": inst["engine"],
                            "ins": [],
                            "name": f"W-{ctr[0]}",
                            "opcode": "NoOp",
                            "outs": [],
                            "sync_info": {"on_update": [], "on_wait": [w]},
                        })
                    si["on_wait"] = [waits[-1]]
            new.append(inst)
        b["instructions"] = new
        for sb in b.get("blocks", []):
            fix_block(sb)

    for f in j.get("functions", []):
        for b in f.get("blocks", []):
            fix_block(b)
    return json.dumps(j).encode()


# Block-major chunk schedule: (jblock, b0, b1, khalf).  khalf selects a
# spatial half of (j, b0) instead of full batches (j1 must be b0+1 then).
CHUNKS = (
    sum([[(j, 0, 4, None), (j, 4, 8, None)] for j in range(6)], [])
    + [(6, 0, 2, None), (6, 2, 4, None), (6, 4, 6, None), (6, 6, 8, None)]
    + [(7, b, b + 1, None) for b in range(7)]
    + [(7, 7, 8, 0), (7, 7, 8, 1)]
)


def _build():
    nc = bass.Bass()
    x = nc.dram_tensor("x", [BPC, NCH, HW], F32, kind="ExternalInput")
    wt = nc.dram_tensor("wt", [IN_DIM, NS, OUT_DIM], F16, kind="ExternalInput")
    # the out-caps axis of v is mathematically degenerate (identical for all
    # o) — the device emits only the unique [b, d] rows; the host unshard
    # step broadcasts to the full [b, o, d] shape.
    out = nc.dram_tensor("out", [BPC, OUT_DIM], F32, kind="ExternalOutput")

    with tile.TileContext(nc) as tc:
        with (
            tc.tile_pool(name="consts", bufs=1) as consts,
            tc.tile_pool(name="xp", bufs=1) as xp,
            tc.tile_pool(name="tp", bufs=1) as tp,
            tc.tile_pool(name="ep", bufs=1) as ep,
            tc.tile_pool(name="pp", bufs=1, space="PSUM") as pp,
        ):
            # constants: rearranged weights WT[i, s, o] = W[s, o, i] / 64,
            # and the group-selector matrix sel[c, g] = (c // 32 == g)
            wt_sb = consts.tile([IN_DIM, NS, OUT_DIM], F16)
            nc.gpsimd.dma_start(out=wt_sb, in_=wt[:])
            sel_sb = consts.tile([128, 4], F32)
            nc.gpsimd.memset(sel_sb, 0.0)
            for g in range(4):
                nc.gpsimd.memset(sel_sb[32 * g:32 * (g + 1), g:g + 1], 1.0)

            # u2[i, b, s] = usum[b, s, i]: accumulated straight out of PE
            u2 = pp.tile([IN_DIM, BPC, NS], F32)
            u2_sb = ep.tile([IN_DIM, BPC, NS], F16)
            sbar_ps = pp.tile([BPC, OUT_DIM], F32)

            # squash scratch
            sq = ep.tile([BPC, OUT_DIM], F32)
            n2 = ep.tile([BPC, 1], F32)
            rr = ep.tile([BPC, 1], F32)
            dd = ep.tile([BPC, 1], F32)
            qq = ep.tile([BPC, 1], F32)
            vrow = ep.tile([BPC, OUT_DIM], F32)

            # xv[p, b, j, m] = x[b, 128*j + p, m]
            xv = x.rearrange("b (j p) m -> p b j m", p=128)

            tks = {}
            for (j, b0, b1, kh) in CHUNKS:
                nb = b1 - b0
                if j not in tks:
                    tks[j] = tp.tile([128, BPC, IN_DIM], F32, tag=f"tk_{j}",
                                     name=f"tk_{j}")
                tk = tks[j]
                if kh is None:
                    xt = xp.tile([128, nb, HW], F32, tag=f"xt_{j}_{b0}",
                                 name=f"xt_{j}_{b0}")
                    nc.sync.dma_start(out=xt, in_=xv[:, b0:b1, j, :])
                    # spatial m = k*32 + i ; reduce over the 8 k-repeats for
                    # all batches of this chunk in one DVE op
                    nc.vector.reduce_sum(
                        out=tk[:, b0:b1, :],
                        in_=xt.rearrange("p b (k i) -> p b i k", i=IN_DIM),
                        axis=mybir.AxisListType.X,
                    )
                    for b in range(b0, b1):
                        # A2: out[i,g] = sum_p tk[p,b,i]*sel[p,g]; s = 4j+g
                        nc.tensor.matmul(
                            out=u2[:, b, 4 * j:4 * j + 4],
                            lhsT=tk[:, b, :],
                            rhs=sel_sb[:],
                            start=True,
                            stop=True,
                            skip_group_check=True,
                        )
                else:
                    # spatial half kh of (block j, batch b0): 4 of the 8
                    # k-repeats; the halves' partials accumulate in PSUM via
                    # the A2 start/stop pair, so the tail reduce is ~194ns
                    xt = xp.tile([128, HW // 2], F32, tag=f"xth_{kh}",
                                 name=f"xth_{kh}")
                    nc.sync.dma_start(
                        out=xt, in_=xv[:, b0, j, kh * 128:(kh + 1) * 128])
                    tkh = tp.tile([128, IN_DIM], F32, tag=f"tkh_{kh}",
                                  name=f"tkh_{kh}")
                    nc.vector.reduce_sum(
                        out=tkh,
                        in_=xt.rearrange("p (k i) -> p i k", i=IN_DIM),
                        axis=mybir.AxisListType.X,
                    )
                    nc.tensor.matmul(
                        out=u2[:, b0, 4 * j:4 * j + 4],
                        lhsT=tkh,
                        rhs=sel_sb[:],
                        start=(kh == 0),
                        stop=(kh == 1),
                        skip_group_check=True,
                    )
                    if kh == 0:
                        continue
                if b1 != BPC:
                    continue
                # all 8 batches of block j are in PSUM: stage to SBUF (f16)
                # and run the weight matmuls so they overlap later DMAs.
                # The copy runs on ACT (idle) — DVE is the tail's pacing
                # engine and must spend every cycle on the reduce stream.
                nc.scalar.copy(
                    out=u2_sb[:, :, 4 * j:4 * j + 4],
                    in_=u2[:, :, 4 * j:4 * j + 4],
                )
                for g in range(4):
                    s = 4 * j + g
                    # B: sbar[b, o] += sum_i usum[b,s,i] * W[s,o,i] / 64
                    nc.tensor.matmul(
                        out=sbar_ps,
                        lhsT=u2_sb[:, :, s],
                        rhs=wt_sb[:, s, :],
                        start=(s == 0),
                        stop=(s == NS - 1),
                        skip_group_check=True,
                    )

            # squash on [8, 64]: v = sbar * coeff, coeff = sqrt(n2)/(1+n2),
            # n2 = |sbar|^2.  (wt is pre-scaled by 1/64 on the host, so
            # sbar_ps IS sbar.)  ACT fuses square+row-sum in one op — the
            # only engine op that squares PSUM data legally (everything else
            # would need two PSUM reads); the sqrt follows on the same
            # queue, in parallel with DVE's add+reciprocal.
            nc.scalar.activation(
                out=sq, in_=sbar_ps,
                func=mybir.ActivationFunctionType.Square,
                accum_out=n2,
            )
            nc.scalar.activation(
                out=rr, in_=n2,
                func=mybir.ActivationFunctionType.Sqrt,
            )
            nc.vector.tensor_scalar_add(out=dd, in0=n2, scalar1=1.0)
            nc.vector.reciprocal(out=qq, in_=dd)
            nc.vector.tensor_scalar(
                out=vrow, in0=sbar_ps,
                scalar1=qq, scalar2=rr,
                op0=AluOpType.mult, op1=AluOpType.mult,
            )
            nc.sync.dma_start(out=out[:], in_=vrow)

    # every compile path (native walrus + bass2jax/axon) serializes via
    # to_json_bytes — splice the single-wait rewrite in there
    orig_to_json = nc.to_json_bytes
    nc.to_json_bytes = lambda: _split_multi_waits(orig_to_json())
    return nc


def kernel(x: np.ndarray, W: np.ndarray, trace: bool = False) -> np.ndarray:
    global LAST_RESULTS, _NC_CACHE
    x = np.ascontiguousarray(np.asarray(x, dtype=np.float32)).reshape(BS, NCH, HW)
    W = np.asarray(W, dtype=np.float32)

    # [i, s, o], pre-scaled so the PE B-stage directly produces sbar; f16 to
    # halve its DMA cost and hit PE's fast matmul path
    wt = (np.ascontiguousarray(W.transpose(2, 0, 1))
          * np.float32(1.0 / 64.0)).astype(np.float16)

    if _NC_CACHE is None:
        _NC_CACHE = _build()
    nc = _NC_CACHE
    in_maps = [
        {"x": np.ascontiguousarray(x[c * BPC:(c + 1) * BPC]), "wt": wt}
        for c in range(N_CORES)
    ]
    res = run_bass_kernel_spmd(nc, in_maps, core_ids=list(range(N_CORES)), trace=trace)
    LAST_RESULTS = res
    rows = np.concatenate([r["out"] for r in res.results], axis=0)  # [64, 64]
    # unshard: materialize the degenerate out-caps axis (v is identical for
    # every o — see the module docstring)
    return np.ascontiguousarray(
        np.broadcast_to(rows[:, None, :], (BS, NUM_OUT, OUT_DIM))
    )


# revision 60
# speedup vs baseline: 1.0278x; 1.0061x over previous
"""Bass/Trainium2 kernel for nn_CapLayer (dynamic-routing capsule layer).

Key algebraic identity (holds for ANY x, W — verified against the reference):
the routing logits b start at zero; softmax over the out-caps axis of an
o-constant tensor is uniform (1/NUM_OUT); with uniform c the squashed v is
identical for every out-cap o, which makes delta_b = <pred, v> o-constant as
well, so b stays o-constant through every routing iteration and the softmax
stays uniform forever.  Hence:

    sbar[b, :] = (1/64) * sum_n pred[b, n, :]
               = (1/64) * sum_{s,i} (sum_p u[b,s,p,i]) * W[s,:,i]
    v[b, o, :] = sbar[b,:] * (|sbar| / (1 + |sbar|^2))     for all o.

So the kernel is: a full reduction of x over the per-group spatial axis
(memory bound — must read all of x exactly once), a tiny matmul with a
rearranged W, a squash, and a broadcast store.  Data-parallel over batch
across 8 cores.

On-chip dataflow per core (8 batches), channel-block (j) major streaming so
the single batched epilogue leaves almost nothing after the last DMA byte:

  - DMA chunks shrink toward the stream tail (halves -> quarters -> singles
    -> spatial halves) so the DVE reduce pipeline drains with the arrivals;
    the final chunk is half a block (64KB, ~194ns reduce).
  - DVE: segmented reduce over the 8 spatial repeats:
    [128c, nb, 256] -> [128c, nb, 32i]
  - PE (A2): per (block j, batch b): lhsT=tk[128c,32i], rhs=sel[128c,4g]
    -> psum u2[32i, b, 4s] (usum lands already transposed, i on partitions)
  - DVE: per block j: copy psum u2 slice -> SBUF as f16
  - PE (B): per group s: lhsT=u2_sb[32i, 8b] (f16), rhs=WT[32i, 64o] (f16)
    accumulating into psum sbar[8b, 64o]; f16 runs PE 4x faster than f32
    and W/usum quantization adds only ~1e-3 relative error.
  - squash epilogue on [8, 64]: Pool fuses square+row-sum (idle engine, no
    accumulator-read penalty), ACT does sqrt, DVE the reciprocal chain and
    the final two-scalar scale; one [8,64] store at the end.

The weights travel as f16 through the SWDGE (Pool) queue: half the DMA
bytes, desc-gen off the 8 HWDGE ring lanes, and no head-of-stream delay.
"""

import json

import numpy as np

import concourse.bass as bass
import concourse.tile as tile
from concourse import mybir
from concourse.alu_op_type import AluOpType
from concourse.bass_utils import run_bass_kernel_spmd

N_CORES = 8
BS = 64
BPC = BS // N_CORES  # 8 batches per core
NCH = 1024           # num_shared * in_dim channels
HW = 256             # 16*16 spatial
NS = 32              # num shared groups
IN_DIM = 32
OUT_DIM = 64
NUM_OUT = 64
F32 = mybir.dt.float32
F16 = mybir.dt.float16

# stash of the last run's BassKernelResults for test harnesses
LAST_RESULTS = None
_NC_CACHE = None


def _split_multi_waits(bir: bytes) -> bytes:
    """The walrus build in this toolchain only accepts a single sync-wait
    command per instruction; Tile freely attaches several (most notably the
    kernel-tail drain, which waits on every outstanding semaphore).  Rewrite
    the BIR so any instruction with N>1 waits is preceded by N-1 single-wait
    NoOps on the same engine — semantically identical (the engine stalls at
    the nops), and acceptable to this codegen."""
    j = json.loads(bir)
    ctr = [0]

    def fix_block(b):
        new = []
        for inst in b.get("instructions", []):
            si = inst.get("sync_info")
            if si:
                waits = si.get("on_wait") or []
                if len(waits) > 1:
                    for w in waits[:-1]:
                        ctr[0] += 1
                        new.append({
                            "debug": inst.get("debug", 0),
                            "engine": inst["engine"],
                            "ins": [],
                            "name": f"W-{ctr[0]}",
                            "opcode": "NoOp",
                            "outs": [],
                            "sync_info": {"on_update": [], "on_wait": [w]},
                        })
                    si["on_wait"] = [waits[-1]]
            new.append(inst)
        b["instructions"] = new
        for sb in b.get("blocks", []):
            fix_block(sb)

    for f in j.get("functions", []):
        for b in f.get("blocks", []):
            fix_block(b)
    return json.dumps(j).encode()


# Block-major chunk schedule: (jblock, b0, b1, khalf).  khalf selects a
# spatial half of (j, b0) instead of full batches (j1 must be b0+1 then).
CHUNKS = (
    sum([[(j, 0, 4, None), (j, 4, 8, None)] for j in range(6)], [])
    + [(6, 0, 2, None), (6, 2, 4, None), (6, 4, 6, None), (6, 6, 8, None)]
    + [(7, b, b + 1, None) for b in range(8)]
)


def _build():
    nc = bass.Bass()
    x = nc.dram_tensor("x", [BPC, NCH, HW], F32, kind="ExternalInput")
    wt = nc.dram_tensor("wt", [IN_DIM, NS, OUT_DIM], F16, kind="ExternalInput")
    # the out-caps axis of v is mathematically degenerate (identical for all
    # o) — the device emits only the unique [b, d] rows; the host unshard
    # step broadcasts to the full [b, o, d] shape.
    out = nc.dram_tensor("out", [BPC, OUT_DIM], F32, kind="ExternalOutput")

    with tile.TileContext(nc) as tc:
        with (
            tc.tile_pool(name="consts", bufs=1) as consts,
            tc.tile_pool(name="xp", bufs=1) as xp,
            tc.tile_pool(name="tp", bufs=1) as tp,
            tc.tile_pool(name="ep", bufs=1) as ep,
            tc.tile_pool(name="pp", bufs=1, space="PSUM") as pp,
        ):
            # constants: rearranged weights WT[i, s, o] = W[s, o, i] / 64,
            # and the group-selector matrix sel[c, g] = (c // 32 == g)
            wt_sb = consts.tile([IN_DIM, NS, OUT_DIM], F16)
            nc.gpsimd.dma_start(out=wt_sb, in_=wt[:])
            sel_sb = consts.tile([128, 4], F32)
            nc.gpsimd.memset(sel_sb, 0.0)
            for g in range(4):
                nc.gpsimd.memset(sel_sb[32 * g:32 * (g + 1), g:g + 1], 1.0)


            # u2[i, b, s] = usum[b, s, i]: accumulated straight out of PE
            u2 = pp.tile([IN_DIM, BPC, NS], F32)
            u2_sb = ep.tile([IN_DIM, BPC, NS], F16)
            sbar_ps = pp.tile([BPC, OUT_DIM], F32)

            # squash scratch
            sbar_sb = ep.tile([BPC, OUT_DIM], F32)
            sq = ep.tile([BPC, OUT_DIM], F32)
            n2 = ep.tile([BPC, 1], F32)
            rr = ep.tile([BPC, 1], F32)
            dd = ep.tile([BPC, 1], F32)
            qq = ep.tile([BPC, 1], F32)
            # [128, 64] because the scatter contract wants the source viewed
            # as 128 partition rows; only rows 0..7 are written/scattered
            vrow128 = ep.tile([128, OUT_DIM], F32)
            vrow = vrow128[0:BPC, :]

            # xv[p, b, j, m] = x[b, 128*j + p, m]
            xv = x.rearrange("b (j p) m -> p b j m", p=128)

            tks = {}
            for (j, b0, b1, kh) in CHUNKS:
                nb = b1 - b0
                if j not in tks:
                    tks[j] = tp.tile([128, BPC, IN_DIM], F32, tag=f"tk_{j}",
                                     name=f"tk_{j}")
                tk = tks[j]
                if kh is None:
                    xt = xp.tile([128, nb, HW], F32, tag=f"xt_{j}_{b0}",
                                 name=f"xt_{j}_{b0}")
                    nc.sync.dma_start(out=xt, in_=xv[:, b0:b1, j, :])
                    # spatial m = k*32 + i ; reduce over the 8 k-repeats for
                    # all batches of this chunk in one DVE op
                    nc.vector.reduce_sum(
                        out=tk[:, b0:b1, :],
                        in_=xt.rearrange("p b (k i) -> p b i k", i=IN_DIM),
                        axis=mybir.AxisListType.X,
                    )
                    for b in range(b0, b1):
                        # A2: out[i,g] = sum_p tk[p,b,i]*sel[p,g]; s = 4j+g
                        nc.tensor.matmul(
                            out=u2[:, b, 4 * j:4 * j + 4],
                            lhsT=tk[:, b, :],
                            rhs=sel_sb[:],
                            start=True,
                            stop=True,
                            skip_group_check=True,
                        )
                else:
                    # spatial half kh of (block j, batch b0): 4 of the 8
                    # k-repeats; the halves' partials accumulate in PSUM via
                    # the A2 start/stop pair, so the tail reduce is ~194ns
                    xt = xp.tile([128, HW // 2], F32, tag=f"xth_{kh}",
                                 name=f"xth_{kh}")
                    nc.sync.dma_start(
                        out=xt, in_=xv[:, b0, j, kh * 128:(kh + 1) * 128])
                    tkh = tp.tile([128, IN_DIM], F32, tag=f"tkh_{kh}",
                                  name=f"tkh_{kh}")
                    nc.vector.reduce_sum(
                        out=tkh,
                        in_=xt.rearrange("p (k i) -> p i k", i=IN_DIM),
                        axis=mybir.AxisListType.X,
                    )
                    nc.tensor.matmul(
                        out=u2[:, b0, 4 * j:4 * j + 4],
                        lhsT=tkh,
                        rhs=sel_sb[:],
                        start=(kh == 0),
                        stop=(kh == 1),
                        skip_group_check=True,
                    )
                    if kh == 0:
                        continue
                if b1 != BPC:
                    continue
                # all 8 batches of block j are in PSUM: stage to SBUF (f16)
                # and run the weight matmuls so they overlap later DMAs.
                # Copies run on ACT (idle) so DVE spends every cycle on the
                # reduce stream — except the last block's copy, which sits on
                # the tail critical path after the reduces are all done, and
                # DVE does it faster.
                if j == 7:
                    nc.vector.tensor_copy(
                        out=u2_sb[:, :, 4 * j:4 * j + 4],
                        in_=u2[:, :, 4 * j:4 * j + 4],
                    )
                else:
                    nc.scalar.copy(
                        out=u2_sb[:, :, 4 * j:4 * j + 4],
                        in_=u2[:, :, 4 * j:4 * j + 4],
                    )
                for g in range(4):
                    s = 4 * j + g
                    # B: sbar[b, o] += sum_i usum[b,s,i] * W[s,o,i] / 64
                    nc.tensor.matmul(
                        out=sbar_ps,
                        lhsT=u2_sb[:, :, s],
                        rhs=wt_sb[:, s, :],
                        start=(s == 0),
                        stop=(s == NS - 1),
                        skip_group_check=True,
                    )

            # squash on [8, 64]: v = sbar * coeff, coeff = sqrt(n2)/(1+n2),
            # n2 = |sbar|^2.  (wt is pre-scaled by 1/64 on the host, so
            # sbar_ps IS sbar.)  One DVE copy stages sbar to SBUF; the fused
            # square+row-sum runs on DVE from that copy (two SBUF reads are
            # legal where two PSUM reads are not, and this skips ACT's 187ns
            # accumulator-read).  ACT's sqrt overlaps DVE's add+reciprocal,
            # and the final scale reads SBUF, nearly twice as fast as PSUM.
            nc.vector.tensor_copy(out=sbar_sb, in_=sbar_ps)
            nc.vector.scalar_tensor_tensor(
                out=sq, in0=sbar_sb, scalar=1.0, in1=sbar_sb,
                op0=AluOpType.mult, op1=AluOpType.mult,
                accum_out=n2,
            )
            nc.scalar.activation(
                out=rr, in_=n2,
                func=mybir.ActivationFunctionType.Sqrt,
            )
            nc.vector.tensor_scalar_add(out=dd, in0=n2, scalar1=1.0)
            nc.vector.reciprocal(out=qq, in_=dd)
            nc.vector.tensor_scalar(
                out=vrow, in0=sbar_sb,
                scalar1=qq, scalar2=rr,
                op0=AluOpType.mult, op1=AluOpType.mult,
            )
            nc.sync.dma_start(out=out[:], in_=vrow)

    # every compile path (native walrus + bass2jax/axon) serializes via
    # to_json_bytes — splice the single-wait rewrite in there
    orig_to_json = nc.to_json_bytes
    nc.to_json_bytes = lambda: _split_multi_waits(orig_to_json())
    return nc


def kernel(x: np.ndarray, W: np.ndarray, trace: bool = False) -> np.ndarray:
    global LAST_RESULTS, _NC_CACHE
    x = np.ascontiguousarray(np.asarray(x, dtype=np.float32)).reshape(BS, NCH, HW)
    W = np.asarray(W, dtype=np.float32)

    # [i, s, o], pre-scaled so the PE B-stage directly produces sbar; f16 to
    # halve its DMA cost and hit PE's fast matmul path
    wt = (np.ascontiguousarray(W.transpose(2, 0, 1))
          * np.float32(1.0 / 64.0)).astype(np.float16)

    if _NC_CACHE is None:
        _NC_CACHE = _build()
    nc = _NC_CACHE
    in_maps = [
        {"x": np.ascontiguousarray(x[c * BPC:(c + 1) * BPC]), "wt": wt}
        for c in range(N_CORES)
    ]
    res = run_bass_kernel_spmd(nc, in_maps, core_ids=list(range(N_CORES)), trace=trace)
    LAST_RESULTS = res
    rows = np.concatenate([r["out"] for r in res.results], axis=0)  # [64, 64]
    # unshard: materialize the degenerate out-caps axis (v is identical for
    # every o — see the module docstring)
    return np.ascontiguousarray(
        np.broadcast_to(rows[:, None, :], (BS, NUM_OUT, OUT_DIM))
    )
